# revision 3
# baseline (speedup 1.0000x reference)
"""CTC loss kernel for Trainium2 (8 NeuronCores, batch-parallel).

Strategy
--------
Batch B=64 is sharded 8 samples/core. Per core, the memory-bound part streams
pred [8,160,6625] f32 once through SBUF in ten [128, 6625] tiles laid out
time-major (partition p = b*16 + t_inner):

  1. DMA tile in (HWDGE, ~3.4 MB)
  2. ScalarE: in-place Exp with fused per-row accumulate -> softmax denominator s
  3. VectorE: r = 1/s
  4. GPSIMD ap_gather: pick the 51 extended-label columns per row (indices are
     per-sample, shared across each 16-partition group)
  5. VectorE scalar_tensor_tensor: p = gathered * r * maskK, where maskK bakes
     in the K=C scale factor and zeroes states beyond each sample's final CTC
     state (2*target_len) - those can never influence the result (transitions
     only move forward in s) and masking them keeps the linear-domain DP in
     f32 range (validated: final-state/max ratio stays >= ~0.4).
  6. SBUF->SBUF DMA regroups partitions (b*16+t) -> per-time-step [8, 51] rows.

The CTC forward recursion then runs on VectorE in the *linear* domain
(probabilities scaled by K, renormalized by the running sum every 8 steps; the
normalizers c_j are written out and folded back on the host in f64):

  alpha_new[s] = (alpha[s] + alpha[s-1] + skip[s]*alpha[s-2]) * p[t, s]

implemented with two guard columns so the shifts are plain free-dim slices.
Outputs per core: final alpha [8, 64] and normalizers [8, 20]. The host
computes -log(alpha[2L] + alpha[2L-1]) + corrections, zero-infinity, the
length division and the batch mean (a 64-element epilogue, f64).
"""

import math
from contextlib import ExitStack

import numpy as np

import concourse.bass as bass
import concourse.tile as tile
from concourse import bacc, mybir
from concourse.bass_utils import run_bass_kernel_spmd

N_CORES = 8
B = 64
T = 160
C = 6625
L = 25
S = 2 * L + 1           # 51 extended states
BPC = B // N_CORES      # 8 samples per core
TBLK = 16               # time steps per streamed tile
NBLK = T // TBLK        # 10 tiles per core
GC = 64                 # gather columns (51 states padded to 64)
NORM_EVERY = 8
NNORM = len([t for t in range(1, T) if t % NORM_EVERY == NORM_EVERY - 1])  # 20
K_SCALE = float(C)

FP = mybir.dt.float32
MULT = mybir.AluOpType.mult
ADD = mybir.AluOpType.add


def build_nc() -> bass.Bass:
    nc = bacc.Bacc("TRN2", target_bir_lowering=False, debug=False,
                   num_devices=N_CORES)
    pred = nc.dram_tensor("pred", [BPC, T, C], FP, kind="ExternalInput")
    idx = nc.dram_tensor("idx", [128, GC // 16], mybir.dt.int16, kind="ExternalInput")
    maskv = nc.dram_tensor("maskv", [BPC, GC], FP, kind="ExternalInput")
    maskk = nc.dram_tensor("maskk", [128, GC], FP, kind="ExternalInput")
    out_alpha = nc.dram_tensor("out_alpha", [BPC, GC], FP, kind="ExternalOutput")
    out_c = nc.dram_tensor("out_c", [BPC, NNORM], FP, kind="ExternalOutput")

    with tile.TileContext(nc) as tc, ExitStack() as ctx:
        pred_pool = ctx.enter_context(tc.tile_pool(name="pred_pool", bufs=3))
        small = ctx.enter_context(tc.tile_pool(name="small", bufs=3))

        def single(shape, dtype, name):
            t, free = tc.tile(shape, dtype, name=name)
            ctx.callback(free)
            return t

        idx_sb = single([128, GC // 16], mybir.dt.int16, "idx_sb")
        maskv_sb = single([BPC, GC], FP, "maskv_sb")
        maskk_sb = single([128, GC], FP, "maskk_sb")
        # ping/pong alpha with 2 guard columns each: ping states at 2..52,
        # pong states at 66..116; guards stay zero forever.
        alpha = single([BPC, 128], FP, "alpha")
        cbuf = single([BPC, NNORM], FP, "cbuf")
        rcn = single([BPC, 1], FP, "rcn")
        pdp = [single([BPC, TBLK, GC], FP, f"pdp{k}") for k in range(NBLK)]
        dram_pool = ctx.enter_context(
            tc.tile_pool(name="pscr_pool", bufs=1, space="DRAM"))
        pscr = [dram_pool.tile([BPC, TBLK, GC], FP, name=f"pscr{k}")
                for k in range(NBLK)]

        nc.sync.dma_start(out=idx_sb[:, :], in_=idx[:, :])
        nc.sync.dma_start(out=maskv_sb[:, :], in_=maskv[:, :])
        nc.sync.dma_start(out=maskk_sb[:, :], in_=maskk[:, :])
        nc.vector.memset(alpha[:, :], 0.0)

        PING, PONG = 0, 64
        jn = 0
        for k in range(NBLK):
            pt = pred_pool.tile([128, C], FP, tag="pt")
            # plain 2D out AP: flat element order of in_ is (b, t, c) row-major,
            # which lands as partition p = b*16 + t  (b-major within the tile)
            # split across both HWDGE queues (sync + scalar) so both DMA
            # engine rings stream pred concurrently
            nc.sync.dma_start(
                out=pt[0:64, :],
                in_=pred[0:4, k * TBLK:(k + 1) * TBLK, :],
            )
            nc.scalar.dma_start(
                out=pt[64:128, :],
                in_=pred[4:8, k * TBLK:(k + 1) * TBLK, :],
            )
            s_k = small.tile([128, 1], FP, tag="s_k")
            nc.scalar.activation(
                out=pt[:, :], in_=pt[:, :],
                func=mybir.ActivationFunctionType.Exp,
                accum_out=s_k[:, :],
            )
            r_k = small.tile([128, 1], FP, tag="r_k")
            nc.vector.reciprocal(r_k[:, :], s_k[:, :])
            g_k = small.tile([128, GC], FP, tag="g_k")
            nc.gpsimd.ap_gather(
                g_k[:, :], pt[:, :], idx_sb[:, :],
                channels=128, num_elems=C, d=1, num_idxs=GC,
            )
            pg_k = small.tile([128, GC], FP, tag="pg_k")
            # (scalar_tensor_tensor / tensor_tensor_reduce crash the DVE exec
            # unit on this runtime - use standard two-op forms instead)
            nc.vector.tensor_scalar_mul(pg_k[:, :], g_k[:, :], r_k[:, 0:1])
            nc.vector.tensor_mul(pg_k[:, :], pg_k[:, :], maskk_sb[:, :])
            # partition regroup (b*16+t, s) -> (b, t, s) via DRAM scratch:
            # both DMAs use plain APs (partition-split SBUF APs miscompile);
            # on the gpsimd SWDGE queue to keep the HWDGE queues free for pred
            nc.gpsimd.dma_start(out=pscr[k][:, :, :], in_=pg_k[:, :])
            nc.gpsimd.dma_start(out=pdp[k][:, :, :], in_=pscr[k][:, :, :])

            for ti in range(TBLK):
                t = k * TBLK + ti
                if t == 0:
                    # alpha0: states 0,1 get p[0, 0:2]
                    nc.vector.tensor_copy(
                        alpha[:, PING + 2:PING + 4], pdp[0][:, 0, 0:2]
                    )
                    continue
                src = PING if t % 2 == 1 else PONG
                dst = PONG if t % 2 == 1 else PING
                vt = small.tile([BPC, S], FP, tag="vt")
                nc.vector.tensor_mul(
                    vt[:, :], alpha[:, src:src + S], maskv_sb[:, 0:S]
                )
                ut = small.tile([BPC, S], FP, tag="ut")
                nc.vector.tensor_add(
                    ut[:, :], alpha[:, src + 2:src + 2 + S],
                    alpha[:, src + 1:src + 1 + S],
                )
                nc.vector.tensor_add(ut[:, :], ut[:, :], vt[:, :])
                pcur = pdp[k][:, ti, 0:S]
                adst = alpha[:, dst + 2:dst + 2 + S]
                if t % NORM_EVERY == NORM_EVERY - 1:
                    nc.vector.tensor_mul(adst, ut[:, :], pcur)
                    nc.vector.tensor_reduce(
                        out=cbuf[:, jn:jn + 1], in_=adst,
                        axis=mybir.AxisListType.X, op=ADD,
                    )
                    nc.vector.reciprocal(rcn[:, :], cbuf[:, jn:jn + 1])
                    nc.vector.tensor_scalar_mul(adst, adst, rcn[:, 0:1])
                    jn += 1
                else:
                    nc.vector.tensor_mul(adst, ut[:, :], pcur)

        assert jn == NNORM
        # final alpha lives in PONG half (t=159 is odd)
        nc.sync.dma_start(out=out_alpha[:, :], in_=alpha[:, PONG:PONG + GC])
        nc.sync.dma_start(out=out_c[:, :], in_=cbuf[:, :])
    nc.compile()
    return nc


_CACHE: dict = {}


def _get_nc() -> bass.Bass:
    if "nc" not in _CACHE:
        _CACHE["nc"] = build_nc()
    return _CACHE["nc"]


LAST_RESULTS = None


def kernel(pred, targets, targets_lengths) -> np.ndarray:
    global LAST_RESULTS
    pred = np.ascontiguousarray(np.asarray(pred, dtype=np.float32))
    targets = np.asarray(targets).astype(np.int64)
    tl = np.asarray(targets_lengths).astype(np.int64)
    assert pred.shape == (B, T, C), pred.shape
    assert targets.shape == (B, L)

    # host prep: extended labels, skip mask, gather indices, band/scale mask
    ext = np.zeros((B, S), dtype=np.int64)
    ext[:, 1::2] = targets
    skip = np.zeros((B, S), dtype=np.float32)
    skip[:, 2:] = ((ext[:, 2:] != 0) & (ext[:, 2:] != ext[:, :-2])).astype(np.float32)

    in_maps = []
    for c in range(N_CORES):
        lo = c * BPC
        idx16 = np.zeros((128, GC // 16), dtype=np.int16)
        maskv = np.zeros((BPC, GC), dtype=np.float32)
        maskk = np.zeros((128, GC), dtype=np.float32)
        for g in range(BPC):
            b = lo + g
            for j in range(S):
                idx16[16 * g + (j % 16), j // 16] = ext[b, j]
            maskv[g, :S] = skip[b]
            band_hi = 2 * int(tl[b])  # states 0..2*tl allowed
            maskk[16 * g:16 * (g + 1), :min(band_hi + 1, S)] = K_SCALE
        in_maps.append({
            "pred": np.ascontiguousarray(pred[lo:lo + BPC]),
            "idx": idx16,
            "maskv": maskv,
            "maskk": maskk,
        })

    nc = _get_nc()
    LAST_RESULTS = run_bass_kernel_spmd(nc, in_maps, core_ids=list(range(N_CORES)))
    results = LAST_RESULTS.results

    # host epilogue (f64, 64 elements)
    logK = math.log(K_SCALE)
    per_sample = np.zeros(B, dtype=np.float64)
    for c in range(N_CORES):
        a = results[c]["out_alpha"].astype(np.float64)   # [8, 64]
        cs = results[c]["out_c"].astype(np.float64)      # [8, 20]
        for g in range(BPC):
            b = c * BPC + g
            a1 = a[g, 2 + 2 * tl[b]]
            a2 = a[g, 2 + 2 * tl[b] - 1]
            tot = a1 + a2
            if tot <= 0.0 or np.any(cs[g] <= 0.0):
                raw = np.inf
            else:
                raw = -(np.log(tot) + np.sum(np.log(cs[g])) - T * logK)
            safe = 0.0 if np.isinf(raw) else raw
            per_sample[b] = safe / max(int(tl[b]), 1)
    return np.asarray(per_sample.mean(), dtype=np.float32)



# revision 4
# speedup vs baseline: 1.2741x; 1.2741x over previous
"""CTC loss kernel for Trainium2 (8 NeuronCores, batch-parallel).

Strategy
--------
Batch B=64 is sharded 8 samples/core. Per core, the memory-bound part streams
pred [8,160,6625] f32 once through SBUF in ten [128, 6625] tiles laid out
time-major (partition p = b*16 + t_inner):

  1. DMA tile in (HWDGE, ~3.4 MB)
  2. ScalarE: in-place Exp with fused per-row accumulate -> softmax denominator s
  3. VectorE: r = 1/s
  4. GPSIMD ap_gather: pick the 51 extended-label columns per row (indices are
     per-sample, shared across each 16-partition group)
  5. VectorE scalar_tensor_tensor: p = gathered * r * maskK, where maskK bakes
     in the K=C scale factor and zeroes states beyond each sample's final CTC
     state (2*target_len) - those can never influence the result (transitions
     only move forward in s) and masking them keeps the linear-domain DP in
     f32 range (validated: final-state/max ratio stays >= ~0.4).
  6. SBUF->SBUF DMA regroups partitions (b*16+t) -> per-time-step [8, 51] rows.

The CTC forward recursion then runs on VectorE in the *linear* domain
(probabilities scaled by K, renormalized by the running sum every 8 steps; the
normalizers c_j are written out and folded back on the host in f64):

  alpha_new[s] = (alpha[s] + alpha[s-1] + skip[s]*alpha[s-2]) * p[t, s]

implemented with two guard columns so the shifts are plain free-dim slices.
Outputs per core: final alpha [8, 64] and normalizers [8, 20]. The host
computes -log(alpha[2L] + alpha[2L-1]) + corrections, zero-infinity, the
length division and the batch mean (a 64-element epilogue, f64).
"""

import math
from contextlib import ExitStack

import numpy as np

import concourse.bass as bass
import concourse.tile as tile
from concourse import bacc, mybir
from concourse.bass_utils import run_bass_kernel_spmd

N_CORES = 8
B = 64
T = 160
C = 6625
L = 25
S = 2 * L + 1           # 51 extended states
BPC = B // N_CORES      # 8 samples per core
TBLK = 16               # time steps per streamed tile
NBLK = T // TBLK        # 10 tiles per core
GC = 64                 # gather columns (51 states padded to 64)
NORM_EVERY = 8
NNORM = len([t for t in range(1, T) if t % NORM_EVERY == NORM_EVERY - 1])  # 20
K_SCALE = float(C)

FP = mybir.dt.float32
MULT = mybir.AluOpType.mult
ADD = mybir.AluOpType.add


def build_nc() -> bass.Bass:
    nc = bacc.Bacc("TRN2", target_bir_lowering=False, debug=False,
                   num_devices=N_CORES)
    pred = nc.dram_tensor("pred", [BPC, T, C], FP, kind="ExternalInput")
    idx = nc.dram_tensor("idx", [128, GC // 16], mybir.dt.int16, kind="ExternalInput")
    maskv = nc.dram_tensor("maskv", [BPC, GC], FP, kind="ExternalInput")
    maskk = nc.dram_tensor("maskk", [128, GC], FP, kind="ExternalInput")
    out_alpha = nc.dram_tensor("out_alpha", [BPC, GC], FP, kind="ExternalOutput")
    out_c = nc.dram_tensor("out_c", [BPC, NNORM], FP, kind="ExternalOutput")

    with tile.TileContext(nc) as tc, ExitStack() as ctx:
        pred_pool = ctx.enter_context(tc.tile_pool(name="pred_pool", bufs=3))
        small = ctx.enter_context(tc.tile_pool(name="small", bufs=3))

        def single(shape, dtype, name):
            t, free = tc.tile(shape, dtype, name=name)
            ctx.callback(free)
            return t

        idx_sb = single([128, GC // 16], mybir.dt.int16, "idx_sb")
        maskv_sb = single([BPC, GC], FP, "maskv_sb")
        maskk_sb = single([128, GC], FP, "maskk_sb")
        # ping/pong alpha with 2 guard columns each: ping states at 2..52,
        # pong states at 66..116; guards stay zero forever.
        alpha = single([BPC, 128], FP, "alpha")
        cbuf = single([BPC, NNORM], FP, "cbuf")
        rcn = single([BPC, 1], FP, "rcn")
        pdp = [single([BPC, TBLK, GC], FP, f"pdp{k}") for k in range(NBLK)]
        dram_pool = ctx.enter_context(
            tc.tile_pool(name="pscr_pool", bufs=1, space="DRAM"))
        pscr = [dram_pool.tile([BPC, TBLK, GC], FP, name=f"pscr{k}")
                for k in range(NBLK)]

        nc.sync.dma_start(out=idx_sb[:, :], in_=idx[:, :])
        nc.sync.dma_start(out=maskv_sb[:, :], in_=maskv[:, :])
        nc.sync.dma_start(out=maskk_sb[:, :], in_=maskk[:, :])
        nc.vector.memset(alpha[:, :], 0.0)

        PING, PONG = 0, 64
        jn = 0
        for k in range(NBLK):
            pt = pred_pool.tile([128, C], FP, tag="pt")
            # plain 2D out AP: flat element order of in_ is (b, t, c) row-major,
            # which lands as partition p = b*16 + t  (b-major within the tile)
            # alternate whole tiles across both HWDGE queues (sync + scalar)
            # so both queue dispatchers stream pred concurrently
            qeng = nc.sync if k % 2 == 0 else nc.scalar
            qeng.dma_start(
                out=pt[:, :],
                in_=pred[:, k * TBLK:(k + 1) * TBLK, :],
            )
            s_k = small.tile([128, 1], FP, tag="s_k")
            nc.scalar.activation(
                out=pt[:, :], in_=pt[:, :],
                func=mybir.ActivationFunctionType.Exp,
                accum_out=s_k[:, :],
            )
            r_k = small.tile([128, 1], FP, tag="r_k")
            nc.vector.reciprocal(r_k[:, :], s_k[:, :])
            g_k = small.tile([128, GC], FP, tag="g_k")
            nc.gpsimd.ap_gather(
                g_k[:, :], pt[:, :], idx_sb[:, :],
                channels=128, num_elems=C, d=1, num_idxs=GC,
            )
            pg_k = small.tile([128, GC], FP, tag="pg_k")
            # (scalar_tensor_tensor / tensor_tensor_reduce crash the DVE exec
            # unit on this runtime - use standard two-op forms instead)
            nc.vector.tensor_scalar_mul(pg_k[:, :], g_k[:, :], r_k[:, 0:1])
            nc.vector.tensor_mul(pg_k[:, :], pg_k[:, :], maskk_sb[:, :])
            # partition regroup (b*16+t, s) -> (b, t, s) via DRAM scratch:
            # both DMAs use plain APs (partition-split SBUF APs miscompile);
            # on the gpsimd SWDGE queue to keep the HWDGE queues free for pred
            nc.gpsimd.dma_start(out=pscr[k][:, :, :], in_=pg_k[:, :])
            nc.gpsimd.dma_start(out=pdp[k][:, :, :], in_=pscr[k][:, :, :])

            for ti in range(TBLK):
                t = k * TBLK + ti
                if t == 0:
                    # alpha0: states 0,1 get p[0, 0:2]
                    nc.vector.tensor_copy(
                        alpha[:, PING + 2:PING + 4], pdp[0][:, 0, 0:2]
                    )
                    continue
                src = PING if t % 2 == 1 else PONG
                dst = PONG if t % 2 == 1 else PING
                vt = small.tile([BPC, S], FP, tag="vt")
                nc.vector.tensor_mul(
                    vt[:, :], alpha[:, src:src + S], maskv_sb[:, 0:S]
                )
                ut = small.tile([BPC, S], FP, tag="ut")
                nc.vector.tensor_add(
                    ut[:, :], alpha[:, src + 2:src + 2 + S],
                    alpha[:, src + 1:src + 1 + S],
                )
                nc.vector.tensor_add(ut[:, :], ut[:, :], vt[:, :])
                pcur = pdp[k][:, ti, 0:S]
                adst = alpha[:, dst + 2:dst + 2 + S]
                if t % NORM_EVERY == NORM_EVERY - 1:
                    nc.vector.tensor_mul(adst, ut[:, :], pcur)
                    nc.vector.tensor_reduce(
                        out=cbuf[:, jn:jn + 1], in_=adst,
                        axis=mybir.AxisListType.X, op=ADD,
                    )
                    nc.vector.reciprocal(rcn[:, :], cbuf[:, jn:jn + 1])
                    nc.vector.tensor_scalar_mul(adst, adst, rcn[:, 0:1])
                    jn += 1
                else:
                    nc.vector.tensor_mul(adst, ut[:, :], pcur)

        assert jn == NNORM
        # final alpha lives in PONG half (t=159 is odd)
        nc.sync.dma_start(out=out_alpha[:, :], in_=alpha[:, PONG:PONG + GC])
        nc.sync.dma_start(out=out_c[:, :], in_=cbuf[:, :])
    nc.compile()
    return nc


_CACHE: dict = {}


def _get_nc() -> bass.Bass:
    if "nc" not in _CACHE:
        _CACHE["nc"] = build_nc()
    return _CACHE["nc"]


LAST_RESULTS = None


def kernel(pred, targets, targets_lengths) -> np.ndarray:
    global LAST_RESULTS
    pred = np.ascontiguousarray(np.asarray(pred, dtype=np.float32))
    targets = np.asarray(targets).astype(np.int64)
    tl = np.asarray(targets_lengths).astype(np.int64)
    assert pred.shape == (B, T, C), pred.shape
    assert targets.shape == (B, L)

    # host prep: extended labels, skip mask, gather indices, band/scale mask
    ext = np.zeros((B, S), dtype=np.int64)
    ext[:, 1::2] = targets
    skip = np.zeros((B, S), dtype=np.float32)
    skip[:, 2:] = ((ext[:, 2:] != 0) & (ext[:, 2:] != ext[:, :-2])).astype(np.float32)

    in_maps = []
    for c in range(N_CORES):
        lo = c * BPC
        idx16 = np.zeros((128, GC // 16), dtype=np.int16)
        maskv = np.zeros((BPC, GC), dtype=np.float32)
        maskk = np.zeros((128, GC), dtype=np.float32)
        for g in range(BPC):
            b = lo + g
            for j in range(S):
                idx16[16 * g + (j % 16), j // 16] = ext[b, j]
            maskv[g, :S] = skip[b]
            band_hi = 2 * int(tl[b])  # states 0..2*tl allowed
            maskk[16 * g:16 * (g + 1), :min(band_hi + 1, S)] = K_SCALE
        in_maps.append({
            "pred": np.ascontiguousarray(pred[lo:lo + BPC]),
            "idx": idx16,
            "maskv": maskv,
            "maskk": maskk,
        })

    nc = _get_nc()
    LAST_RESULTS = run_bass_kernel_spmd(nc, in_maps, core_ids=list(range(N_CORES)))
    results = LAST_RESULTS.results

    # host epilogue (f64, 64 elements)
    logK = math.log(K_SCALE)
    per_sample = np.zeros(B, dtype=np.float64)
    for c in range(N_CORES):
        a = results[c]["out_alpha"].astype(np.float64)   # [8, 64]
        cs = results[c]["out_c"].astype(np.float64)      # [8, 20]
        for g in range(BPC):
            b = c * BPC + g
            a1 = a[g, 2 + 2 * tl[b]]
            a2 = a[g, 2 + 2 * tl[b] - 1]
            tot = a1 + a2
            if tot <= 0.0 or np.any(cs[g] <= 0.0):
                raw = np.inf
            else:
                raw = -(np.log(tot) + np.sum(np.log(cs[g])) - T * logK)
            safe = 0.0 if np.isinf(raw) else raw
            per_sample[b] = safe / max(int(tl[b]), 1)
    return np.asarray(per_sample.mean(), dtype=np.float32)

